# revision 4
# baseline (speedup 1.0000x reference)
"""Multi-head distance (attention) layer on 8 TRN2 NeuronCores.

Sharding: data-parallel over batch. B=8 -> one batch element per core.
Each core computes a full multi-head self-attention for its [L=1024, D=256]
slice with H=8 heads of dim 64. No collectives needed.

The ScalarE exp stream is the pacing engine (~55us of pure streaming at
1 col/cycle @1.2GHz is unavoidable for the 8M softmax elements per core),
so the kernel is organized to keep ScalarE 100% exp:
  - x is shipped pre-transposed (xT) and pre-pos-encoded (qkT = xT + peT),
    both host-side layout prep, so there is no on-device transpose stage,
    no pos-enc add, and all input DMA descriptors are 2KB.
  - input DMAs are spread over the SP/DVE/Pool queues with the S-critical
    tensors (qkT, wk, bq, wq) ahead of the V-path ones (xT, wv).
  - all PSUM drains run on DVE; kTz zero-fills and the exp-table preload
    memset run on GpSimd; ScalarE issues nothing but exp activations.
  - S chunks are packed into [128, 1536] PSUM tiles (3 banks, 2 bufs) so
    the 128 S-chunks need only 43 exp instructions.
Per-core algorithm (all matmul operands fp16; fp16 has fp32-grade mantissa
for this problem's value ranges):
  qT   = Wq.T @ qkT + bq        per 128-row block, bias added in the drain
  kTz  = Wk.T @ qkT             per-head tiles, other head's rows zeroed
                                (so S contracts K=128; K=64 runs half-rate)
  v    = xT.T @ Wv              [m, j] blocks, 65th column of ones appended
  per head h:
    sT[m,l] = sum_d kTz[d,m] qT[d,l]     matmul, K=128 (zero-masked)
    eT      = exp(0.125 * sT)            ScalarE, PSUM->SBUF, fp16
    O[l,d]+Z = eT.T @ [v_h | 1]          matmul, 4 column-groups per PSUM bank
    out_h   = O * (1/Z)                  DVE reciprocal + broadcast multiply
The S/exp stream is software-pipelined (S(h+1) halves interleave with O(h)
quads) so the in-order PE queue never blocks the ScalarE exp stream.
Bias handling: bq added during the qT drain; bk only shifts score rows by a
constant (softmax-invariant) so it is dropped; bv shifts the output by
exactly repeat(bv, 64) because softmax rows sum to 1, added on the host.
Output is written fp16 (halves the 2MB out DMA) and upcast on the host.
"""

import numpy as np
import ml_dtypes

import concourse.bass as bass
import concourse.mybir as mybir
import concourse.tile as tile
from concourse import bacc
from concourse.bass_utils import run_bass_kernel_spmd

B, L, D = 8, 1024, 256
H, HD = 8, 64
J = H * HD  # 512
TEMPERATURE = 10000.0

f32 = mybir.dt.float32
bf16 = mybir.dt.float16  # fp16: same PE rate as bf16, 8x the mantissa

_CACHE = {}
LAST_RESULT = None  # BassKernelResults of the most recent run (for profiling)
TRACE = False

STILE = 1536  # S-chunk PSUM/exp tile width (3 chunks of 512 = 3 banks)


def _emit(tc, aps):
    nc = tc.nc
    Exp = mybir.ActivationFunctionType.Exp
    xt, qkt, wq, wk, wv, bqc, out = (
        aps["xt"], aps["qkt"], aps["wq"], aps["wk"], aps["wv"], aps["bqc"],
        aps["out"],
    )

    xtr = xt.rearrange("(t p) l -> t p l", p=128)        # [2, 128, 1024]
    qktr = qkt.rearrange("(t p) l -> t p l", p=128)      # [2, 128, 1024]
    wqr = wq.rearrange("(t p) j -> t p j", p=128)        # [2, 128, 512]
    wkr = wk.rearrange("(t p) j -> t p j", p=128)
    wvr = wv.rearrange("(t p) j -> t p j", p=128)
    outr = out.rearrange("(n p) j -> p n j", p=128)      # [128, 8, 512]

    import contextlib
    ctx = contextlib.ExitStack()
    persist = ctx.enter_context(tc.tile_pool(name="persist", bufs=1))
    epool = ctx.enter_context(tc.tile_pool(name="epool", bufs=16))
    rpool = ctx.enter_context(tc.tile_pool(name="rpool", bufs=4))
    s_ps = ctx.enter_context(tc.tile_pool(name="sps", bufs=2, space="PSUM"))
    o_ps = ctx.enter_context(tc.tile_pool(name="ops", bufs=2, space="PSUM"))

    # --- input DMAs, spread over 3 queues, S-critical tensors first ---
    qkT = [persist.tile([128, 1024], bf16, name=f"qkT{t}") for t in range(2)]
    xT = [persist.tile([128, 1024], bf16, name=f"xT{t}") for t in range(2)]
    w_sb = {
        wname: [persist.tile([128, 512], bf16, name=f"{wname}_sb{t}")
                for t in range(2)]
        for wname in ("wq", "wk", "wv")
    }
    bq_sb = persist.tile([128, 4], f32, name="bq_sb")

    nc.sync.dma_start(out=bq_sb[:], in_=bqc[:, :])
    nc.sync.dma_start(out=qkT[0][:], in_=qktr[0])
    nc.sync.dma_start(out=w_sb["wk"][0][:], in_=wkr[0])
    nc.sync.dma_start(out=xT[0][:], in_=xtr[0])
    nc.scalar.dma_start(out=qkT[1][:], in_=qktr[1])
    nc.scalar.dma_start(out=w_sb["wk"][1][:], in_=wkr[1])
    nc.gpsimd.dma_start(out=w_sb["wq"][0][:], in_=wqr[0])
    nc.gpsimd.dma_start(out=w_sb["wq"][1][:], in_=wqr[1])
    nc.gpsimd.dma_start(out=xT[1][:], in_=xtr[1])
    nc.gpsimd.dma_start(out=w_sb["wv"][0][:], in_=wvr[0])
    nc.gpsimd.dma_start(out=w_sb["wv"][1][:], in_=wvr[1])

    # --- ACT exp-table preload (off the attention critical path) ---
    sc_in = persist.tile([128, 8], f32, name="sc_in")
    sc_out = persist.tile([128, 8], f32, name="sc_out")
    nc.gpsimd.memset(sc_in[:], 0.0)
    nc.scalar.activation(sc_out[:], sc_in[:], Exp)

    kTz = [persist.tile([128, 1024], bf16, name=f"kTz{h}") for h in range(8)]
    for h in range(8):
        nc.gpsimd.memset(kTz[h][:], 0.0)

    # --- QKV projections (PSUM fills share the "s" tag slots) ---
    qT = [persist.tile([128, 1024], bf16, name=f"qT{j}") for j in range(4)]
    v_sb = [persist.tile([128, 8, 65], bf16, name=f"v_sb{m}") for m in range(8)]
    for m in range(8):
        nc.gpsimd.memset(v_sb[m][:, :, 64:65], 1.0)

    def qk_piece(j, which, l2):
        wname = "wq" if which == "q" else "wk"
        pq = s_ps.tile([128, 512], f32, tag="s", name="pq")
        for c2 in range(2):
            nc.tensor.matmul(
                pq[:, 0:512],
                lhsT=w_sb[wname][c2][:, j * 128:(j + 1) * 128],
                rhs=qkT[c2][:, l2 * 512:(l2 + 1) * 512],
                start=(c2 == 0),
                stop=(c2 == 1),
            )
        dsl = slice(l2 * 512, (l2 + 1) * 512)
        if which == "q":
            nc.vector.tensor_scalar_add(
                qT[j][:, dsl], pq[:, 0:512], bq_sb[:, j:j + 1]
            )
        else:
            nc.vector.tensor_copy(kTz[2 * j][0:64, dsl], pq[0:64, 0:512])
            nc.vector.tensor_copy(kTz[2 * j + 1][64:128, dsl], pq[64:128, 0:512])

    def v_proj(m):
        pv = s_ps.tile([128, 512], f32, tag="s", name="pv")
        for c2 in range(2):
            nc.tensor.matmul(
                pv[:, 0:512],
                lhsT=xT[c2][:, m * 128:(m + 1) * 128],
                rhs=w_sb["wv"][c2][:],
                start=(c2 == 0),
                stop=(c2 == 1),
            )
        nc.vector.tensor_copy(
            v_sb[m][:, :, 0:64], pv[:, 0:512].rearrange("p (h d) -> p h d", h=8)
        )

    # --- attention: S-chunks packed into [128, STILE] PSUM tiles; one exp
    # per tile. Software-pipelined: S(h+1) emitted before O(h). ---
    out_sb = persist.tile([128, 8, 512], bf16, name="out_sb")
    epos = {}  # (h, mc, l2) -> (e_tile, col_offset)
    state = {"tile": None, "off": 0, "chunks": []}

    def flush_exp():
        if state["tile"] is None or not state["chunks"]:
            return
        e = epool.tile([128, state["off"]], bf16, tag="e", name="e")
        nc.scalar.activation(
            e[:], state["tile"][:, 0:state["off"]], Exp, scale=float(HD) ** -0.5
        )
        for key, off in state["chunks"]:
            epos[key] = (e, off)
        state["tile"] = None
        state["off"] = 0
        state["chunks"] = []

    def s_chunk(h, mc, l2):
        if state["tile"] is None:
            state["tile"] = s_ps.tile([128, STILE], f32, tag="s", name="ps")
        off = state["off"]
        nc.tensor.matmul(
            state["tile"][:, off:off + 512],
            lhsT=kTz[h][:, mc * 128:(mc + 1) * 128],
            rhs=qT[h // 2][:, l2 * 512:(l2 + 1) * 512],
            start=True,
            stop=True,
        )
        state["chunks"].append(((h, mc, l2), off))
        state["off"] = off + 512
        if state["off"] == STILE:
            flush_exp()

    def emit_S_half(h, l2):
        for mc in range(8):
            s_chunk(h, mc, l2)

    def emit_O_quad(h, q):
        hsl = slice(h * 64, (h + 1) * 64)
        pO = o_ps.tile([128, 260], f32, tag="o", name="pO")
        for g in range(4):
            lc = 4 * q + g
            l2, sub = lc // 4, lc % 4
            for mc in range(8):
                e, off = epos[(h, mc, l2)]
                nc.tensor.matmul(
                    pO[:, 65 * g:65 * g + 65],
                    lhsT=e[:, off + sub * 128:off + (sub + 1) * 128],
                    rhs=v_sb[mc][:, h, :],
                    start=(mc == 0),
                    stop=(mc == 7),
                )
        pOr = pO.rearrange("p (g c) -> p g c", g=4)      # [128, 4, 65]
        rc = rpool.tile([128, 4], f32, tag="rc", name="rc")
        nc.vector.reciprocal(rc[:], pOr[:, :, 64])
        rcb = bass.AP(
            tensor=rc.tensor, offset=rc.offset,
            ap=[rc.ap[0], rc.ap[1], [0, 64]],
        )
        nc.vector.tensor_mul(
            out_sb[:, 4 * q:4 * q + 4, hsl], pOr[:, :, 0:64], rcb
        )
        if h == 7:
            engs = [nc.sync, nc.gpsimd, nc.scalar, nc.sync]
            for g2 in range(4):
                sl2 = slice(4 * q + g2, 4 * q + g2 + 1)
                engs[g2].dma_start(out=outr[:, sl2, hsl], in_=out_sb[:, sl2, hsl])
        else:
            eng = nc.sync if q == 0 else nc.gpsimd
            eng.dma_start(
                out=outr[:, 4 * q:4 * q + 4, hsl],
                in_=out_sb[:, 4 * q:4 * q + 4, hsl],
            )

    # schedule: (head, half) S-emissions and (head, quad) O-emissions are
    # interleaved one step apart; QKV projections dropped in just before the
    # first S-half that needs them. V only feeds O so it comes after S(0).
    qk_piece(0, "k", 0)
    qk_piece(0, "k", 1)
    qk_piece(0, "q", 0)
    emit_S_half(0, 0)
    qk_piece(0, "q", 1)
    for m in range(4):
        v_proj(m)
    emit_S_half(0, 1)
    for m in range(4, 8):
        v_proj(m)
    # qk pieces for projection j are spread across the 4 steps of head block
    # 2j-1 so they never bunch up in front of an S-fill.
    inject = {
        (1, i): (1, w, l2) for i, (w, l2) in enumerate(
            [("q", 0), ("q", 1), ("k", 0), ("k", 1)])
    }
    inject.update({(3, i): (2, w, l2) for i, (w, l2) in enumerate(
        [("q", 0), ("q", 1), ("k", 0), ("k", 1)])})
    inject.update({(5, i): (3, w, l2) for i, (w, l2) in enumerate(
        [("q", 0), ("q", 1), ("k", 0), ("k", 1)])})
    for h in range(1, 8):
        for stepi, (kind, hh, part) in enumerate(
            [("S", h, 0), ("O", h - 1, 0), ("S", h, 1), ("O", h - 1, 1)]
        ):
            if kind == "S":
                emit_S_half(hh, part)
            else:
                emit_O_quad(hh, part)
            if (h, stepi) in inject:
                j, w, l2 = inject[(h, stepi)]
                qk_piece(j, w, l2)
    emit_O_quad(7, 0)
    flush_exp()
    emit_O_quad(7, 1)
    ctx.close()


def _build():
    if "nc" in _CACHE:
        return _CACHE["nc"]
    nc = bacc.Bacc("TRN2", target_bir_lowering=False, debug=False, num_devices=8)
    aps = {
        "xt": nc.dram_tensor("xt", [D, L], bf16, kind="ExternalInput").ap(),
        "qkt": nc.dram_tensor("qkt", [D, L], bf16, kind="ExternalInput").ap(),
        "wq": nc.dram_tensor("wq", [D, J], bf16, kind="ExternalInput").ap(),
        "wk": nc.dram_tensor("wk", [D, J], bf16, kind="ExternalInput").ap(),
        "wv": nc.dram_tensor("wv", [D, J], bf16, kind="ExternalInput").ap(),
        "bqc": nc.dram_tensor("bqc", [128, 4], f32, kind="ExternalInput").ap(),
        "out": nc.dram_tensor("out", [L, J], bf16, kind="ExternalOutput").ap(),
    }
    with tile.TileContext(nc) as tc:
        _emit(tc, aps)
    nc.compile()
    _CACHE["nc"] = nc
    return nc


def _pe_T():
    embed = np.arange(L, dtype=np.float32)
    dim_t = np.arange(D, dtype=np.float32)
    dim_t = (np.float32(TEMPERATURE) ** (2.0 * np.floor(dim_t / 2.0) / np.float32(D))).astype(np.float32)
    pos = embed[:, None] / dim_t  # [L, D]
    pe = np.stack([np.sin(pos[:, 0::2]), np.cos(pos[:, 1::2])], axis=2).reshape(L, D)
    return np.ascontiguousarray(pe.T.astype(np.float32))  # [D, L]


def kernel(**inputs):
    global LAST_RESULT
    bf = np.float16
    x = np.asarray(inputs["x"], dtype=np.float32)
    wq = np.ascontiguousarray(np.asarray(inputs["Wq"], dtype=np.float32).astype(bf))
    wk = np.ascontiguousarray(np.asarray(inputs["Wk"], dtype=np.float32).astype(bf))
    wv = np.ascontiguousarray(np.asarray(inputs["Wv"], dtype=np.float32).astype(bf))
    bq = np.asarray(inputs["bq"], dtype=np.float32)
    bv = np.asarray(inputs["bv"], dtype=np.float32)

    nc = _build()
    bqc = np.ascontiguousarray(np.repeat(bq, HD).reshape(4, 128).T)  # [128, 4]
    peT = _pe_T()                                                    # [D, L]
    xT = np.swapaxes(x, 1, 2)                                        # [B, D, L]
    qkT = (xT + peT[None]).astype(bf)                                # [B, D, L]
    xT = xT.astype(bf)
    base = {"wq": wq, "wk": wk, "wv": wv, "bqc": bqc}
    in_maps = [
        {**base, "xt": np.ascontiguousarray(xT[b]),
         "qkt": np.ascontiguousarray(qkT[b])}
        for b in range(B)
    ]
    res = run_bass_kernel_spmd(
        nc, in_maps, core_ids=list(range(B)), trace=TRACE
    )
    LAST_RESULT = res
    out = np.stack([res.results[b]["out"] for b in range(B)]).astype(np.float32)
    out += np.repeat(bv, HD)[None, None, :]
    return out


# revision 8
# speedup vs baseline: 1.0572x; 1.0572x over previous
"""Multi-head distance (attention) layer on 8 TRN2 NeuronCores.

Sharding: data-parallel over batch. B=8 -> one batch element per core.
Each core computes a full multi-head self-attention for its [L=1024, D=256]
slice with H=8 heads of dim 64. No collectives needed.

The ScalarE exp stream is the pacing engine (~55us of pure streaming at
1 col/cycle @1.2GHz is unavoidable for the 8M softmax elements per core),
so the kernel is organized to keep ScalarE 100% exp:
  - x is shipped pre-transposed (xT) and pre-pos-encoded (qkT = xT + peT),
    both host-side layout prep, so there is no on-device transpose stage,
    no pos-enc add, and all input DMA descriptors are 2KB.
  - input DMAs are spread over the SP/DVE/Pool queues with the S-critical
    tensors (qkT, wk, bq, wq) ahead of the V-path ones (xT, wv).
  - all PSUM drains run on DVE; kTz zero-fills and the exp-table preload
    memset run on GpSimd; ScalarE issues nothing but exp activations.
  - S chunks are packed into [128, 1536] PSUM tiles (3 banks, 2 bufs) so
    the 128 S-chunks need only 43 exp instructions.
Per-core algorithm (all matmul operands fp16; fp16 has fp32-grade mantissa
for this problem's value ranges):
  qT   = Wq.T @ qkT + bq        per 128-row block, bias added in the drain
  kTz  = Wk.T @ qkT             per-head tiles, other head's rows zeroed
                                (so S contracts K=128; K=64 runs half-rate)
  v    = xT.T @ Wv              [m, j] blocks, 65th column of ones appended
  per head h:
    sT[m,l] = sum_d kTz[d,m] qT[d,l]     matmul, K=128 (zero-masked)
    eT      = exp(0.125 * sT)            ScalarE, PSUM->SBUF, fp16
    O[l,d]+Z = eT.T @ [v_h | 1]          matmul, 4 column-groups per PSUM bank
    out_h   = O * (1/Z)                  DVE reciprocal + broadcast multiply
The S/exp stream is software-pipelined (S(h+1) halves interleave with O(h)
quads) so the in-order PE queue never blocks the ScalarE exp stream.
Bias handling: bq added during the qT drain; bk only shifts score rows by a
constant (softmax-invariant) so it is dropped; bv shifts the output by
exactly repeat(bv, 64) because softmax rows sum to 1, added on the host.
Output is written fp16 (halves the 2MB out DMA) and upcast on the host.
"""

import numpy as np
import ml_dtypes

import concourse.bass as bass
import concourse.mybir as mybir
import concourse.tile as tile
from concourse import bacc
from concourse.bass_utils import run_bass_kernel_spmd

B, L, D = 8, 1024, 256
H, HD = 8, 64
J = H * HD  # 512
TEMPERATURE = 10000.0

f32 = mybir.dt.float32
bf16 = mybir.dt.float16  # fp16: same PE rate as bf16, 8x the mantissa

_CACHE = {}
LAST_RESULT = None  # BassKernelResults of the most recent run (for profiling)
TRACE = False

STILE = 1536  # S-chunk PSUM/exp tile width (3 chunks of 512 = 3 banks)


def _emit(tc, aps):
    nc = tc.nc
    Exp = mybir.ActivationFunctionType.Exp
    xt, qkt, wq, wk, wv, bqc, out = (
        aps["xt"], aps["qkt"], aps["wq"], aps["wk"], aps["wv"], aps["bqc"],
        aps["out"],
    )

    xtr = xt.rearrange("(t p) l -> t p l", p=128)        # [2, 128, 1024]
    qktr = qkt.rearrange("(t p) l -> t p l", p=128)      # [2, 128, 1024]
    wqr = wq.rearrange("(t p) j -> t p j", p=128)        # [2, 128, 512]
    wkr = wk.rearrange("(t p) j -> t p j", p=128)
    wvr = wv.rearrange("(t p) j -> t p j", p=128)
    outr = out.rearrange("(n p) j -> p n j", p=128)      # [128, 8, 512]

    import contextlib
    ctx = contextlib.ExitStack()
    persist = ctx.enter_context(tc.tile_pool(name="persist", bufs=1))
    epool = ctx.enter_context(tc.tile_pool(name="epool", bufs=16))
    rpool = ctx.enter_context(tc.tile_pool(name="rpool", bufs=4))
    s_ps = ctx.enter_context(tc.tile_pool(name="sps", bufs=2, space="PSUM"))
    o_ps = ctx.enter_context(tc.tile_pool(name="ops", bufs=2, space="PSUM"))

    # --- input DMAs, spread over 3 queues, S-critical tensors first ---
    qkT = [persist.tile([128, 1024], bf16, name=f"qkT{t}") for t in range(2)]
    xT = [persist.tile([128, 1024], bf16, name=f"xT{t}") for t in range(2)]
    w_sb = {
        wname: [persist.tile([128, 512], bf16, name=f"{wname}_sb{t}")
                for t in range(2)]
        for wname in ("wq", "wk", "wv")
    }
    bq_sb = persist.tile([128, 4], f32, name="bq_sb")

    nc.sync.dma_start(out=bq_sb[:], in_=bqc[:, :])
    nc.sync.dma_start(out=qkT[0][:], in_=qktr[0])
    nc.sync.dma_start(out=w_sb["wk"][0][:], in_=wkr[0])
    nc.sync.dma_start(out=xT[0][:], in_=xtr[0])
    nc.scalar.dma_start(out=qkT[1][:], in_=qktr[1])
    nc.scalar.dma_start(out=w_sb["wk"][1][:], in_=wkr[1])
    nc.gpsimd.dma_start(out=w_sb["wq"][0][:], in_=wqr[0])
    nc.gpsimd.dma_start(out=w_sb["wq"][1][:], in_=wqr[1])
    nc.gpsimd.dma_start(out=xT[1][:], in_=xtr[1])
    nc.gpsimd.dma_start(out=w_sb["wv"][0][:], in_=wvr[0])
    nc.gpsimd.dma_start(out=w_sb["wv"][1][:], in_=wvr[1])

    # --- ACT exp-table preload (off the attention critical path); memsets
    # on DVE, which is idle until the first PSUM drain ---
    sc_in = persist.tile([128, 8], f32, name="sc_in")
    sc_out = persist.tile([128, 8], f32, name="sc_out")
    nc.vector.memset(sc_in[:], 0.0)
    nc.scalar.activation(sc_out[:], sc_in[:], Exp)

    kTz = [persist.tile([128, 1024], bf16, name=f"kTz{h}") for h in range(8)]
    for h in range(8):
        nc.vector.memset(kTz[h][:], 0.0)

    # --- QKV projections (PSUM fills share the "s" tag slots) ---
    qT = [persist.tile([128, 1024], bf16, name=f"qT{j}") for j in range(4)]
    v_sb = [persist.tile([128, 8, 65], bf16, name=f"v_sb{m}") for m in range(8)]
    for m in range(8):
        nc.vector.memset(v_sb[m][:, :, 64:65], 1.0)

    def qk_piece(j, which, l2):
        wname = "wq" if which == "q" else "wk"
        pq = s_ps.tile([128, 512], f32, tag="s", name="pq")
        for c2 in range(2):
            nc.tensor.matmul(
                pq[:, 0:512],
                lhsT=w_sb[wname][c2][:, j * 128:(j + 1) * 128],
                rhs=qkT[c2][:, l2 * 512:(l2 + 1) * 512],
                start=(c2 == 0),
                stop=(c2 == 1),
            )
        dsl = slice(l2 * 512, (l2 + 1) * 512)
        if which == "q":
            nc.vector.tensor_scalar_add(
                qT[j][:, dsl], pq[:, 0:512], bq_sb[:, j:j + 1]
            )
        else:
            nc.vector.tensor_copy(kTz[2 * j][0:64, dsl], pq[0:64, 0:512])
            nc.vector.tensor_copy(kTz[2 * j + 1][64:128, dsl], pq[64:128, 0:512])

    def v_proj(m):
        pv = s_ps.tile([128, 512], f32, tag="s", name="pv")
        for c2 in range(2):
            nc.tensor.matmul(
                pv[:, 0:512],
                lhsT=xT[c2][:, m * 128:(m + 1) * 128],
                rhs=w_sb["wv"][c2][:],
                start=(c2 == 0),
                stop=(c2 == 1),
            )
        nc.vector.tensor_copy(
            v_sb[m][:, :, 0:64], pv[:, 0:512].rearrange("p (h d) -> p h d", h=8)
        )

    # --- attention: S-chunks packed into [128, STILE] PSUM tiles; one exp
    # per tile. Software-pipelined: S(h+1) emitted before O(h). ---
    out_sb = persist.tile([128, 8, 512], bf16, name="out_sb")
    epos = {}  # (h, mc, l2) -> (e_tile, col_offset)
    state = {"tile": None, "off": 0, "chunks": []}

    def flush_exp():
        if state["tile"] is None or not state["chunks"]:
            return
        e = epool.tile([128, state["off"]], bf16, tag="e", name="e")
        nc.scalar.activation(
            e[:], state["tile"][:, 0:state["off"]], Exp, scale=float(HD) ** -0.5
        )
        for key, off in state["chunks"]:
            epos[key] = (e, off)
        state["tile"] = None
        state["off"] = 0
        state["chunks"] = []

    def s_chunk(h, mc, l2):
        if state["tile"] is None:
            state["tile"] = s_ps.tile([128, STILE], f32, tag="s", name="ps")
        off = state["off"]
        nc.tensor.matmul(
            state["tile"][:, off:off + 512],
            lhsT=kTz[h][:, mc * 128:(mc + 1) * 128],
            rhs=qT[h // 2][:, l2 * 512:(l2 + 1) * 512],
            start=True,
            stop=True,
        )
        state["chunks"].append(((h, mc, l2), off))
        state["off"] = off + 512
        if state["off"] == STILE:
            flush_exp()

    def emit_S_half(h, l2, filler=None):
        # 8 chunks flushed 3+3+2 (two 1536-col exps + one 1024-col exp), so
        # every half ends tile-aligned: no partially-filled PSUM tile ever
        # waits behind the filler work, and the exp stream never straddles.
        # The filler (an O-quad or projection pieces) sits in the slot where
        # the PE would otherwise stall on the s_ps buf-rotation dependency.
        for mc in range(6):
            s_chunk(h, mc, l2)
        if filler is not None:
            filler()
        for mc in range(6, 8):
            s_chunk(h, mc, l2)
        flush_exp()

    def emit_O_quad(h, q):
        hsl = slice(h * 64, (h + 1) * 64)
        pO = o_ps.tile([128, 260], f32, tag="o", name="pO")
        for g in range(4):
            lc = 4 * q + g
            l2, sub = lc // 4, lc % 4
            for mc in range(8):
                e, off = epos[(h, mc, l2)]
                nc.tensor.matmul(
                    pO[:, 65 * g:65 * g + 65],
                    lhsT=e[:, off + sub * 128:off + (sub + 1) * 128],
                    rhs=v_sb[mc][:, h, :],
                    start=(mc == 0),
                    stop=(mc == 7),
                )
        pOr = pO.rearrange("p (g c) -> p g c", g=4)      # [128, 4, 65]
        rc = rpool.tile([128, 4], f32, tag="rc", name="rc")
        nc.vector.reciprocal(rc[:], pOr[:, :, 64])
        rcb = bass.AP(
            tensor=rc.tensor, offset=rc.offset,
            ap=[rc.ap[0], rc.ap[1], [0, 64]],
        )
        nc.vector.tensor_mul(
            out_sb[:, 4 * q:4 * q + 4, hsl], pOr[:, :, 0:64], rcb
        )
        if h == 7:
            # n-rows 4q..4q+4 now have all 8 heads: one contiguous 1KB-per-
            # line DMA for the full j-range instead of 8 strided 128B ones.
            eng = nc.sync if q == 0 else nc.scalar
            eng.dma_start(out=outr[:, 4 * q:4 * q + 4, :],
                          in_=out_sb[:, 4 * q:4 * q + 4, :])

    # schedule: each half carries its own filler; O(h-1) quads ride inside
    # h's halves, projections for head-pair j inside head-block 2j-1.
    def mk(*fns):
        def run():
            for f in fns:
                f()
        return run

    qk_piece(0, "k", 0)
    qk_piece(0, "k", 1)
    qk_piece(0, "q", 0)
    emit_S_half(0, 0, mk(lambda: qk_piece(0, "q", 1),
                         lambda: v_proj(0), lambda: v_proj(1)))
    emit_S_half(0, 1, mk(*(lambda m=m: v_proj(m) for m in range(2, 6))))
    # head h's halves carry O(h-1) quads plus, in blocks 1/3/5, the q-pieces
    # for projection j=(h+1)//2; the matching k-pieces go right after the
    # block's second half so kTz[2j] is ready before head 2j starts.
    extra = {
        (1, 0): [lambda: v_proj(6), lambda: v_proj(7)],
        (1, 1): [lambda: qk_piece(1, "q", 0), lambda: qk_piece(1, "q", 1)],
        (3, 0): [lambda: qk_piece(2, "q", 0)],
        (3, 1): [lambda: qk_piece(2, "q", 1)],
        (5, 0): [lambda: qk_piece(3, "q", 0)],
        (5, 1): [lambda: qk_piece(3, "q", 1)],
    }
    post = {
        (1, 1): [lambda: qk_piece(1, "k", 0), lambda: qk_piece(1, "k", 1)],
        (3, 1): [lambda: qk_piece(2, "k", 0), lambda: qk_piece(2, "k", 1)],
        (5, 1): [lambda: qk_piece(3, "k", 0), lambda: qk_piece(3, "k", 1)],
    }
    for h in range(1, 8):
        for l2 in range(2):
            fns = extra.get((h, l2), []) + [
                lambda hh=h, qq=l2: emit_O_quad(hh - 1, qq)
            ]
            emit_S_half(h, l2, mk(*fns))
            for fn in post.get((h, l2), []):
                fn()
    emit_O_quad(7, 0)
    emit_O_quad(7, 1)
    ctx.close()


def _build():
    if "nc" in _CACHE:
        return _CACHE["nc"]
    nc = bacc.Bacc("TRN2", target_bir_lowering=False, debug=False, num_devices=8)
    aps = {
        "xt": nc.dram_tensor("xt", [D, L], bf16, kind="ExternalInput").ap(),
        "qkt": nc.dram_tensor("qkt", [D, L], bf16, kind="ExternalInput").ap(),
        "wq": nc.dram_tensor("wq", [D, J], bf16, kind="ExternalInput").ap(),
        "wk": nc.dram_tensor("wk", [D, J], bf16, kind="ExternalInput").ap(),
        "wv": nc.dram_tensor("wv", [D, J], bf16, kind="ExternalInput").ap(),
        "bqc": nc.dram_tensor("bqc", [128, 4], f32, kind="ExternalInput").ap(),
        "out": nc.dram_tensor("out", [L, J], bf16, kind="ExternalOutput").ap(),
    }
    with tile.TileContext(nc) as tc:
        _emit(tc, aps)
    nc.compile()
    _CACHE["nc"] = nc
    return nc


def _pe_T():
    embed = np.arange(L, dtype=np.float32)
    dim_t = np.arange(D, dtype=np.float32)
    dim_t = (np.float32(TEMPERATURE) ** (2.0 * np.floor(dim_t / 2.0) / np.float32(D))).astype(np.float32)
    pos = embed[:, None] / dim_t  # [L, D]
    pe = np.stack([np.sin(pos[:, 0::2]), np.cos(pos[:, 1::2])], axis=2).reshape(L, D)
    return np.ascontiguousarray(pe.T.astype(np.float32))  # [D, L]


def kernel(**inputs):
    global LAST_RESULT
    bf = np.float16
    x = np.asarray(inputs["x"], dtype=np.float32)
    wq = np.ascontiguousarray(np.asarray(inputs["Wq"], dtype=np.float32).astype(bf))
    wk = np.ascontiguousarray(np.asarray(inputs["Wk"], dtype=np.float32).astype(bf))
    wv = np.ascontiguousarray(np.asarray(inputs["Wv"], dtype=np.float32).astype(bf))
    bq = np.asarray(inputs["bq"], dtype=np.float32)
    bv = np.asarray(inputs["bv"], dtype=np.float32)

    nc = _build()
    bqc = np.ascontiguousarray(np.repeat(bq, HD).reshape(4, 128).T)  # [128, 4]
    peT = _pe_T()                                                    # [D, L]
    xT = np.swapaxes(x, 1, 2)                                        # [B, D, L]
    qkT = (xT + peT[None]).astype(bf)                                # [B, D, L]
    xT = xT.astype(bf)
    base = {"wq": wq, "wk": wk, "wv": wv, "bqc": bqc}
    in_maps = [
        {**base, "xt": np.ascontiguousarray(xT[b]),
         "qkt": np.ascontiguousarray(qkT[b])}
        for b in range(B)
    ]
    res = run_bass_kernel_spmd(
        nc, in_maps, core_ids=list(range(B)), trace=TRACE
    )
    LAST_RESULT = res
    out = np.stack([res.results[b]["out"] for b in range(B)]).astype(np.float32)
    out += np.repeat(bv, HD)[None, None, :]
    return out
